# revision 21
# baseline (speedup 1.0000x reference)
"""Causal self-attention kernel for Trainium2, sharded over 8 NeuronCores.

Problem (hardcoded): x [2, 2048, 1024] fp32, Wq/Wk/Wv/Wo [1024, 1024], bo [1024].
H = 16 heads, head dim 64.

Sharding: batch x head-group hybrid. Core i handles batch i//4 and head group
i%4 (4 heads = 256 features, processed as 2 pairs of 2 heads). Each core
computes its partial out-projection y_i = ctx_i @ Wo[rows_i] for its batch;
the host sums the 4 partials per batch (the "all-reduce") and adds bo.
vs pure head-parallel TP this halves per-core x-in / y-out DMA and the
PSUM->SBUF output copy traffic.

On-device layout (per core, T = 2048 tokens of its batch):

  x           loaded chunk-wise ([128, 512] e-tiles, double-buffered) as xT
  qT, kT      [128 (2 heads x 64 d), cols] feature-major from W.T-slices as
              lhsT, xT as rhs; q is per-chunk, k accumulates over the batch
  v           token-major DIRECTLY from projection (lhsT = xT tile, rhs = Wv
              slice) -> [t, d]; no PE transposes. Stored as [k, (vA|1|vB|1)]
              so ctx row 64 accumulates the softmax denominator for free
  scoresT     [k-tile 128, 2 heads, q-chunk 512] in a 2-bank PSUM tile
  expT        exp(scoresT / 8) for both heads in ONE ScalarE activation;
              causal-masked on DVE via a triangular 0/1 mask
  ctxT (+l)   [65, 512] per head; normalized via reciprocal (DVE) +
              partition_broadcast (GPSIMD) + mul (DVE)
  y partial   lhsT = normalized ctxT t-tile per pair, rhs = Wo local rows,
              2 accumulating matmuls (one per pair). Out-projection of chunk
              qc is emitted after the projections of chunk qc+1 so the
              normalize latency never stalls the PE queue.

PSUM (8 banks): sc 2x2 + ctx 2x1 + mm (psq/psk/vps/yp ring) 2x1.
"""

import sys

import numpy as np

try:
    import concourse.bass as bass  # noqa: F401
except ImportError:  # harness environments without concourse on sys.path
    sys.path.insert(0, "/opt/trn_rl_repo")
    import concourse.bass as bass  # noqa: F401

from contextlib import ExitStack

import concourse.mybir as mybir
import concourse.tile as tile
from concourse import bacc
from concourse.bass import ts
from concourse.bass_utils import run_bass_kernel_spmd

F32 = mybir.dt.float32
F32R = mybir.dt.float32r
F16 = mybir.dt.float16

N_CORES = 8
B, S, E = 2, 2048, 1024
H, D = 16, 64
NB = 2            # batch shards
NG = 4            # head-group shards (4 heads each)
NP = 2            # head pairs per core
EL = 128          # features per pair (2 heads x 64)
CH = 512          # q-chunk width (one PSUM bank of fp32)
KT = 128          # k-tile width
NE = E // 128     # e-tiles in the contraction dim


def build_attention(batch=B, seq=S, dt_in=F32, dt_out=F32, n_reps=1):
    """Build the per-core Bass program (same program on all 8 cores)."""
    import os
    ablate = os.environ.get("ABLATE", "none")  # timing experiments only
    ncb = seq // CH            # q-chunks (per-core batch)
    nkt_b = seq // KT          # k-tiles

    nc = bacc.Bacc("TRN2", debug=False, num_devices=N_CORES)

    dt_ind = F32R if dt_in == F32 else dt_in
    dt_i = dt_ind
    xT = nc.dram_tensor("xT", [E, seq], dt_ind, kind="ExternalInput").ap()
    wq = nc.dram_tensor("wq", [NP * 128, E], dt_ind, kind="ExternalInput").ap()
    wk = nc.dram_tensor("wk", [NP * 128, E], dt_ind, kind="ExternalInput").ap()
    wv = nc.dram_tensor("wv", [NP * 128, E], dt_ind, kind="ExternalInput").ap()
    wo = nc.dram_tensor("wo", [NP * 128, E], dt_ind, kind="ExternalInput").ap()
    # trineg[q, k] = -30000 iff k > q (strict upper): rank-128 causal mask
    # added into the scores PSUM group via lhsT=trineg, rhs=iden
    trineg = nc.dram_tensor("trineg", [128, 128], dt_ind, kind="ExternalInput").ap()
    iden = nc.dram_tensor("iden", [128, 128], dt_ind, kind="ExternalInput").ap()
    onesc = nc.dram_tensor("onesc", [128, 1], dt_ind, kind="ExternalInput").ap()
    y = nc.dram_tensor("y", [seq, E], dt_out, kind="ExternalOutput").ap()

    with tile.TileContext(nc) as tc, ExitStack() as ctx, \
            nc.allow_low_precision(reason="fp16 internals validated vs reference"):
        consts = ctx.enter_context(tc.tile_pool(name="consts", bufs=1))
        xt_pool = ctx.enter_context(tc.tile_pool(name="xt", bufs=2 * NE))
        big = ctx.enter_context(tc.tile_pool(name="big", bufs=1))
        q_pool = ctx.enter_context(tc.tile_pool(name="qp", bufs=2))
        ex_pool = ctx.enter_context(tc.tile_pool(name="ex", bufs=6))
        cn_pool = ctx.enter_context(tc.tile_pool(name="cn", bufs=4))
        ysb_pool = ctx.enter_context(tc.tile_pool(name="ysb", bufs=4))
        small = ctx.enter_context(tc.tile_pool(name="small", bufs=3))
        # PSUM (8 banks): sc 2x2 + ctx 2x1 + mm 2x1
        sc_pool = ctx.enter_context(tc.tile_pool(name="scps", bufs=2, space="PSUM"))
        ctx_pool = ctx.enter_context(tc.tile_pool(name="ctxps", bufs=2, space="PSUM"))
        mm_pool = ctx.enter_context(tc.tile_pool(name="mmps", bufs=2, space="PSUM"))

        # ---- constants / weights (resident, loaded once per dispatch) ----
        wq_sb, wk_sb, wv_sb, wo_sb = [], [], [], []
        for p in range(NP):
            for lst, src, nm in ((wq_sb, wq, "wq"), (wk_sb, wk, "wk"),
                                 (wv_sb, wv, "wv"), (wo_sb, wo, "wo")):
                t = consts.tile([128, E], dt_ind, name=f"{nm}{p}_sb")
                nc.sync.dma_start(t[:], src[p * 128:(p + 1) * 128, :])
                lst.append(t)
        tri_sb = consts.tile([128, 128], dt_ind, name="tri_sb")
        nc.sync.dma_start(tri_sb[:], trineg)
        id_sb = consts.tile([128, 128], dt_ind, name="id_sb")
        nc.sync.dma_start(id_sb[:], iden)
        ones_sb = consts.tile([128, 1], dt_ind, name="ones_sb")
        nc.sync.dma_start(ones_sb[:], onesc)
        if ablate == "noexp":
            exc_sb = consts.tile([128, 2, CH], dt_i, name="exc_sb")
            nc.vector.memset(exc_sb[:], 1.0)

        rep_cm = tc.For_i(0, n_reps, 1) if n_reps > 1 else None
        if rep_cm is not None:
            rep_cm.__enter__()

        # persistent per-pair K / V state for the whole batch
        kTs, vsbs = [], []
        for p in range(NP):
            kT = big.tile([128, seq], dt_i, tag=f"kT{p}", name=f"kT{p}", bufs=1)
            vsb = big.tile([128, nkt_b, 130], dt_i, tag=f"vsb{p}",
                           name=f"vsb{p}", bufs=1)
            nc.vector.tensor_copy(
                vsb.rearrange("p t (h c) -> p (t h) c", c=65)[:, :, 64:65],
                ones_sb.broadcast_to([128, 2 * nkt_b, 1]))
            kTs.append(kT)
            vsbs.append(vsb)

        def alloc_x(qc):
            xts = []
            for e in range(NE):
                xt = xt_pool.tile([128, CH], dt_ind, tag="xt", name="xt")
                nc.sync.dma_start(xt[:], xT[e * 128:(e + 1) * 128,
                                            qc * CH:(qc + 1) * CH])
                xts.append(xt)
            return xts

        def norm_fillers(cps, ctxn):
            """Normalize ctxT[0:64] / l (row 64) for both heads."""
            def f():
                for hi in range(2):
                    rc = small.tile([1, CH], F32, tag="rc", name="rc")
                    nc.vector.reciprocal(rc[:], cps[hi][64:65, :])
                    bc = small.tile([64, CH], F32, tag="bc", name="bc")
                    nc.gpsimd.partition_broadcast(bc[:], rc[:])
                    nc.vector.tensor_mul(
                        ctxn[hi * 64:(hi + 1) * 64, :], cps[hi][0:64, :], bc[:])
            return [(0, f)]

        def proj_fillers(qc, p, xts):
            """q/k/v projection of (qc, p) as (cost_ns, closure) filler units.
            Tiles are allocated now (ring order = FIFO pop order)."""
            psq = mm_pool.tile([128, CH], F32, tag="mm", name="psq")
            psk = mm_pool.tile([128, CH], F32, tag="mm", name="psk")
            qTt = q_pool.tile([128, CH], dt_i, tag="qT", name="qTt")
            fillers = []
            for ps, w_sb, done in ((psq, wq_sb[p], None), (psk, wk_sb[p], None)):
                for e0 in range(0, NE, 2):
                    def f(ps=ps, w_sb=w_sb, e0=e0):
                        for e in (e0, e0 + 1):
                            nc.tensor.matmul(ps[:], w_sb[:, ts(e, 128)],
                                             xts[e][:],
                                             start=(e == 0), stop=(e == NE - 1))
                    fillers.append((426, f))

            def fq():
                nc.vector.tensor_copy(qTt[:], psq[:])
            fillers.append((0, fq))

            def fk():
                nc.vector.tensor_copy(kTs[p][:, ts(qc, CH)], psk[:])
            fillers.append((0, fk))

            for j in range(CH // 128):
                tt = qc * (CH // 128) + j
                vps = mm_pool.tile([128, CH], F32, tag="mm", name="vps")

                def fv1(j=j, vps=vps):
                    for e in range(NE // 2):
                        nc.tensor.matmul(vps[:, 0:128], xts[e][:, ts(j, 128)],
                                         wv_sb[p][:, ts(e, 128)],
                                         start=(e == 0), stop=False)
                fillers.append((213, fv1))

                def fv2(j=j, tt=tt, vps=vps):
                    for e in range(NE // 2, NE):
                        nc.tensor.matmul(vps[:, 0:128], xts[e][:, ts(j, 128)],
                                         wv_sb[p][:, ts(e, 128)],
                                         start=False, stop=(e == NE - 1))
                    nc.vector.tensor_copy(
                        vsbs[p][:, tt, 0:130].rearrange(
                            "p (h c) -> p h c", h=2)[:, :, 0:64],
                        vps[:, 0:128].rearrange("p (h c) -> p h c", h=2))
                fillers.append((213, fv2))
            return fillers, qTt

        def outproj_fillers(qc, ctxns, lo=0, hi=CH // 128):
            fillers = []
            for j in range(lo, hi):
                tt = qc * (CH // 128) + j
                for eo in range(E // CH):
                    yp = mm_pool.tile([128, CH], F32, tag="mm", name="yp")

                    def f1(j=j, eo=eo, yp=yp):
                        for p in range(NP):
                            nc.tensor.matmul(yp[:], ctxns[p][:, ts(j, 128)],
                                             wo_sb[p][:, ts(eo, CH)],
                                             start=(p == 0), stop=(p == NP - 1))
                    fillers.append((426, f1))

                    def f2(tt=tt, eo=eo, yp=yp):
                        ysb = ysb_pool.tile([128, CH], dt_out, tag="ysb",
                                            name="ysb")
                        nc.vector.tensor_copy(ysb[:], yp[:])
                        if ablate != "noydma":
                            nc.sync.dma_start(
                                y[tt * 128:(tt + 1) * 128,
                                  eo * CH:(eo + 1) * CH], ysb[:])
                    fillers.append((0, f2))
            return fillers

        def attention(qc, p, qTt, fillers):
            """Attention over k-tiles, popping filler work between tiles to
            keep the PE busy through the ACT-bound exp stream."""
            nkt = (qc * CH + CH) // KT
            cps = [ctx_pool.tile([65, CH], F32, tag="ctx", name=f"ctx_ps{hi}")
                   for hi in range(2)]

            def emit_ctx(kt, ex, c0, n):
                for hi in range(2):
                    nc.tensor.matmul(
                        cps[hi][:, c0:CH],
                        vsbs[p][:, kt, hi * 65:(hi + 1) * 65],
                        ex[:, hi, 0:n],
                        start=(kt == 0), stop=(kt == nkt - 1),
                        skip_group_check=True,
                    )

            deficit = 1200.0  # warmup: first exp latency to hide
            filled = 0.0
            prev_ctx = None
            for kt in range(nkt):
                c0 = max(0, kt * KT - qc * CH)
                n = CH - c0
                diag = kt * KT >= qc * CH
                sc = sc_pool.tile([128, 2, CH], F32, tag="sc", name="sc_ps")
                for hi in range(2):
                    r0 = hi * 64
                    nc.tensor.matmul(
                        sc[:, hi, 0:n],
                        kTs[p][r0:r0 + 64, ts(kt, KT)],
                        qTt[r0:r0 + 64, c0:CH],
                        start=True, stop=not diag,
                        skip_group_check=diag,
                    )
                    if diag:
                        # add -30000 to k>q entries of the 128-col diagonal
                        # block; exp then underflows to 0
                        nc.tensor.matmul(
                            sc[:, hi, 0:128], tri_sb[:], id_sb[:],
                            start=False, stop=True,
                            skip_group_check=True,
                        )
                ex = ex_pool.tile([128, 2, CH], dt_i, tag="ex", name="ex")
                nex = 8 if ablate == "tinyexp" else n  # timing-only ablation
                nc.scalar.activation(
                    ex[:, :, 0:nex], sc[:, :, 0:nex],
                    mybir.ActivationFunctionType.Exp, scale=1.0 / np.sqrt(D))
                # ACT-vs-PE deficit for this tile: exp+overheads minus the
                # scores/ctx matmul time; pop that much filler PE work
                deficit += (2 * n * 0.833 + 356) - (4 * n * 0.4167)
                while fillers and filled < deficit:
                    cost, f = fillers.pop(0)
                    f()
                    filled += cost
                # delay ctx by one k-tile: scores(kt+1) issues on the PE
                # before ctx(kt), hiding the exp latency
                if prev_ctx is not None:
                    emit_ctx(*prev_ctx)
                prev_ctx = (kt, ex, c0, n)
            emit_ctx(*prev_ctx)
            for _, f in fillers:  # drain leftovers
                f()
            ctxn = cn_pool.tile([128, CH], dt_i, tag="ctxn", name="ctxn")
            return cps, ctxn

        pairs = [(qc, p) for qc in range(ncb) for p in range(NP)]
        xts_by = {0: alloc_x(0)}
        first, qTt_next = proj_fillers(0, 0, xts_by[0])
        for _, f in first:
            f()
        pending = None       # ctxn tiles of prev chunk awaiting out-proj
        pending_norm = None  # (cps, ctxn) of prev pair awaiting normalize
        ctxns = []
        for idx, (qc, p) in enumerate(pairs):
            qTt = qTt_next
            fillers = []
            if pending_norm is not None:
                fillers += norm_fillers(*pending_norm)
            if (qc, p) != pairs[-1]:
                nqc, npp = pairs[idx + 1]
                if npp == 0:
                    xts_by[nqc] = alloc_x(nqc)
                    xts_by.pop(nqc - 1, None)
                nf, qTt_next = proj_fillers(nqc, npp, xts_by[nqc])
                fillers += nf
            if pending is not None:
                half = CH // 256  # 2: out-proj t-tiles split across both pairs
                if p == 0:
                    fillers += outproj_fillers(qc - 1, pending, 0, half)
                else:
                    fillers += outproj_fillers(qc - 1, pending, half, CH // 128)
                    pending = None
            cps, ctxn = attention(qc, p, qTt, fillers)
            pending_norm = (cps, ctxn)
            ctxns.append(ctxn)
            if p == NP - 1:
                pending = ctxns
                ctxns = []

        for _, f in norm_fillers(*pending_norm):
            f()
        pending_norm = None
        for _, f in outproj_fillers(ncb - 1, pending):
            f()
        pending = None

        if rep_cm is not None:
            rep_cm.__exit__(None, None, None)

    nc.compile()
    return nc


def _prep_inputs(x, Wq, Wk, Wv, Wo, dt_in=np.float32):
    """Host-side sharding: transpose x per batch, slice weights per core."""
    batch, seq, _ = x.shape
    xTs = [np.ascontiguousarray(x[b].T).astype(dt_in) for b in range(batch)]
    # trineg[q, k] = -30000 iff k > q; used as lhsT with rhs=I so the scores
    # PSUM group accumulates -30000 into causally-invalid diagonal entries
    trineg = -30000.0 * np.triu(np.ones((128, 128), np.float32), 1)
    identity = np.eye(128, dtype=np.float32)

    def warr(w):  # [E, 128] col-slice -> SBUF layout [128, 8*128]
        return np.ascontiguousarray(
            w.reshape(NE, 128, 128).transpose(1, 0, 2).reshape(128, E)
        ).astype(dt_in)

    in_maps = []
    for i in range(N_CORES):
        b, g = i // NG, i % NG
        cs = [slice(g * 256 + p * 128, g * 256 + (p + 1) * 128)
              for p in range(NP)]
        in_maps.append({
            "xT": xTs[b],
            "wq": np.concatenate([warr(Wq[:, c]) for c in cs], axis=0),
            "wk": np.concatenate([warr(Wk[:, c]) for c in cs], axis=0),
            "wv": np.concatenate([warr(Wv[:, c]) for c in cs], axis=0),
            "wo": np.concatenate(
                [np.ascontiguousarray(Wo[c, :]) for c in cs], axis=0
            ).astype(dt_in),
            "trineg": trineg.astype(dt_in),
            "iden": identity.astype(dt_in),
            "onesc": np.ones((128, 1), dt_in),
        })
    return in_maps


_CACHE = {}


def _get_nc(batch, seq, dt_in, dt_out):
    key = (batch, seq, dt_in, dt_out)
    if key not in _CACHE:
        _CACHE[key] = build_attention(batch, seq, dt_in, dt_out)
    return _CACHE[key]


DT_IN = F16   # fp16 x/W transfers; projections accumulate fp32 in PSUM
DT_OUT = F16  # fp16 partial-y transfers; host sums in fp32


def kernel(x, Wq, Wk, Wv, Wo, bo, _trace=False):
    x = np.asarray(x, np.float32)
    batch, seq, _ = x.shape
    nc = _get_nc(batch, seq, DT_IN, DT_OUT)
    in_maps = _prep_inputs(x, np.asarray(Wq), np.asarray(Wk), np.asarray(Wv),
                           np.asarray(Wo),
                           dt_in=np.float16 if DT_IN == F16 else np.float32)
    res = run_bass_kernel_spmd(nc, in_maps, core_ids=list(range(N_CORES)),
                               trace=_trace)
    y = np.empty((batch, seq, E), np.float32)
    bo32 = np.asarray(bo, np.float32)
    for b in range(NB):
        parts = [res.results[b * NG + g]["y"].astype(np.float32)
                 for g in range(NG)]
        y[b] = np.sum(parts, axis=0, dtype=np.float32) + bo32
    if _trace:
        kernel.last_results = res
    return y


# revision 23
# speedup vs baseline: 1.0146x; 1.0146x over previous
"""Causal self-attention kernel for Trainium2, sharded over 8 NeuronCores.

Problem (hardcoded): x [2, 2048, 1024] fp32, Wq/Wk/Wv/Wo [1024, 1024], bo [1024].
H = 16 heads, head dim 64.

Sharding: batch x head-group hybrid. Core i handles batch i//4 and head group
i%4 (4 heads = 256 features, processed as 2 pairs of 2 heads). Each core
computes its partial out-projection y_i = ctx_i @ Wo[rows_i] for its batch;
the host sums the 4 partials per batch (the "all-reduce") and adds bo.
vs pure head-parallel TP this halves per-core x-in / y-out DMA and the
PSUM->SBUF output copy traffic.

On-device layout (per core, T = 2048 tokens of its batch):

  x           loaded chunk-wise ([128, 512] e-tiles, double-buffered) as xT
  qT, kT      [128 (2 heads x 64 d), cols] feature-major from W.T-slices as
              lhsT, xT as rhs; q is per-chunk, k accumulates over the batch
  v           token-major DIRECTLY from projection (lhsT = xT tile, rhs = Wv
              slice) -> [t, d]; no PE transposes. Stored as [k, (vA|1|vB|1)]
              so ctx row 64 accumulates the softmax denominator for free
  scoresT     [k-tile 128, 2 heads, q-chunk 512] in a 2-bank PSUM tile
  expT        exp(scoresT / 8) for both heads in ONE ScalarE activation;
              causal-masked on DVE via a triangular 0/1 mask
  ctxT (+l)   [65, 512] per head; normalized via reciprocal (DVE) +
              partition_broadcast (GPSIMD) + mul (DVE)
  y partial   lhsT = normalized ctxT t-tile per pair, rhs = Wo local rows,
              2 accumulating matmuls (one per pair). Out-projection of chunk
              qc is emitted after the projections of chunk qc+1 so the
              normalize latency never stalls the PE queue.

PSUM (8 banks): sc 2x2 + ctx 2x1 + mm (psq/psk/vps/yp ring) 2x1.
"""

import sys

import numpy as np

try:
    import concourse.bass as bass  # noqa: F401
except ImportError:  # harness environments without concourse on sys.path
    sys.path.insert(0, "/opt/trn_rl_repo")
    import concourse.bass as bass  # noqa: F401

from contextlib import ExitStack

import concourse.mybir as mybir
import concourse.tile as tile
from concourse import bacc
from concourse.bass import ts
from concourse.bass_utils import run_bass_kernel_spmd

F32 = mybir.dt.float32
F32R = mybir.dt.float32r
F16 = mybir.dt.float16

N_CORES = 8
B, S, E = 2, 2048, 1024
H, D = 16, 64
NB = 2            # batch shards
NG = 4            # head-group shards (4 heads each)
NP = 2            # head pairs per core
EL = 128          # features per pair (2 heads x 64)
CH = 512          # q-chunk width (one PSUM bank of fp32)
KT = 128          # k-tile width
NE = E // 128     # e-tiles in the contraction dim


def build_attention(batch=B, seq=S, dt_in=F32, dt_out=F32, n_reps=1):
    """Build the per-core Bass program (same program on all 8 cores)."""
    import os
    ablate = os.environ.get("ABLATE", "none")  # timing experiments only
    ncb = seq // CH            # q-chunks (per-core batch)
    nkt_b = seq // KT          # k-tiles

    nc = bacc.Bacc("TRN2", debug=False, num_devices=N_CORES)

    dt_ind = F32R if dt_in == F32 else dt_in
    dt_i = dt_ind
    xT = nc.dram_tensor("xT", [E, seq], dt_ind, kind="ExternalInput").ap()
    wq = nc.dram_tensor("wq", [NP * 128, E], dt_ind, kind="ExternalInput").ap()
    wk = nc.dram_tensor("wk", [NP * 128, E], dt_ind, kind="ExternalInput").ap()
    wv = nc.dram_tensor("wv", [NP * 128, E], dt_ind, kind="ExternalInput").ap()
    wo = nc.dram_tensor("wo", [NP * 128, E], dt_ind, kind="ExternalInput").ap()
    # trineg[q, k] = -30000 iff k > q (strict upper): rank-128 causal mask
    # added into the scores PSUM group via lhsT=trineg, rhs=iden
    trineg = nc.dram_tensor("trineg", [128, 128], dt_ind, kind="ExternalInput").ap()
    iden = nc.dram_tensor("iden", [128, 128], dt_ind, kind="ExternalInput").ap()
    onesc = nc.dram_tensor("onesc", [128, 1], dt_ind, kind="ExternalInput").ap()
    y = nc.dram_tensor("y", [seq, E], dt_out, kind="ExternalOutput").ap()

    with tile.TileContext(nc) as tc, ExitStack() as ctx, \
            nc.allow_low_precision(reason="fp16 internals validated vs reference"):
        consts = ctx.enter_context(tc.tile_pool(name="consts", bufs=1))
        xt_pool = ctx.enter_context(tc.tile_pool(name="xt", bufs=2 * NE))
        big = ctx.enter_context(tc.tile_pool(name="big", bufs=1))
        q_pool = ctx.enter_context(tc.tile_pool(name="qp", bufs=2))
        ex_pool = ctx.enter_context(tc.tile_pool(name="ex", bufs=6))
        cn_pool = ctx.enter_context(tc.tile_pool(name="cn", bufs=4))
        ysb_pool = ctx.enter_context(tc.tile_pool(name="ysb", bufs=4))
        small = ctx.enter_context(tc.tile_pool(name="small", bufs=3))
        # PSUM (8 banks): sc 2x2 + ctx 2x1 + mm 2x1
        sc_pool = ctx.enter_context(tc.tile_pool(name="scps", bufs=2, space="PSUM"))
        ctx_pool = ctx.enter_context(tc.tile_pool(name="ctxps", bufs=2, space="PSUM"))
        mm_pool = ctx.enter_context(tc.tile_pool(name="mmps", bufs=2, space="PSUM"))

        # ---- constants / weights (resident, loaded once per dispatch) ----
        wq_sb, wk_sb, wv_sb, wo_sb = [], [], [], []
        for p in range(NP):
            for lst, src, nm in ((wq_sb, wq, "wq"), (wk_sb, wk, "wk"),
                                 (wv_sb, wv, "wv"), (wo_sb, wo, "wo")):
                t = consts.tile([128, E], dt_ind, name=f"{nm}{p}_sb")
                nc.sync.dma_start(t[:], src[p * 128:(p + 1) * 128, :])
                lst.append(t)
        tri_sb = consts.tile([128, 128], dt_ind, name="tri_sb")
        nc.sync.dma_start(tri_sb[:], trineg)
        id_sb = consts.tile([128, 128], dt_ind, name="id_sb")
        nc.sync.dma_start(id_sb[:], iden)
        ones_sb = consts.tile([128, 1], dt_ind, name="ones_sb")
        nc.sync.dma_start(ones_sb[:], onesc)
        if ablate == "decouple":
            exc_sb = consts.tile([128, 2, CH], dt_i, name="exc_sb")
            nc.vector.memset(exc_sb[:], 1.0)

        rep_cm = tc.For_i(0, n_reps, 1) if n_reps > 1 else None
        if rep_cm is not None:
            rep_cm.__enter__()

        # persistent per-pair K / V state for the whole batch
        kTs, vsbs = [], []
        for p in range(NP):
            kT = big.tile([128, seq], dt_i, tag=f"kT{p}", name=f"kT{p}", bufs=1)
            vsb = big.tile([128, nkt_b, 130], dt_i, tag=f"vsb{p}",
                           name=f"vsb{p}", bufs=1)
            nc.vector.tensor_copy(
                vsb.rearrange("p t (h c) -> p (t h) c", c=65)[:, :, 64:65],
                ones_sb.broadcast_to([128, 2 * nkt_b, 1]))
            kTs.append(kT)
            vsbs.append(vsb)

        def alloc_x(qc):
            xts = []
            for e in range(NE):
                xt = xt_pool.tile([128, CH], dt_ind, tag="xt", name="xt")
                nc.sync.dma_start(xt[:], xT[e * 128:(e + 1) * 128,
                                            qc * CH:(qc + 1) * CH])
                xts.append(xt)
            return xts

        def norm_fillers(cps, ctxn):
            """Normalize ctxT[0:64] / l (row 64) for both heads."""
            def f():
                for hi in range(2):
                    rc = small.tile([1, CH], F32, tag="rc", name="rc")
                    nc.vector.reciprocal(rc[:], cps[hi][64:65, :])
                    bc = small.tile([64, CH], F32, tag="bc", name="bc")
                    nc.gpsimd.partition_broadcast(bc[:], rc[:])
                    nc.vector.tensor_mul(
                        ctxn[hi * 64:(hi + 1) * 64, :], cps[hi][0:64, :], bc[:])
            return [(0, f)]

        def proj_fillers(qc, p, xts):
            """q/k/v projection of (qc, p) as (cost_ns, closure) filler units.
            Tiles are allocated now (ring order = FIFO pop order)."""
            psq = mm_pool.tile([128, CH], F32, tag="mm", name="psq")
            psk = mm_pool.tile([128, CH], F32, tag="mm", name="psk")
            qTt = q_pool.tile([128, CH], dt_i, tag="qT", name="qTt")
            fillers = []
            for ps, w_sb, done in ((psq, wq_sb[p], None), (psk, wk_sb[p], None)):
                for e0 in range(0, NE, 2):
                    def f(ps=ps, w_sb=w_sb, e0=e0):
                        for e in (e0, e0 + 1):
                            nc.tensor.matmul(ps[:], w_sb[:, ts(e, 128)],
                                             xts[e][:],
                                             start=(e == 0), stop=(e == NE - 1))
                    fillers.append((426, f))

            def fq():
                nc.vector.tensor_copy(qTt[:], psq[:])
            fillers.append((0, fq))

            def fk():
                nc.vector.tensor_copy(kTs[p][:, ts(qc, CH)], psk[:])
            fillers.append((0, fk))

            for j in range(CH // 128):
                tt = qc * (CH // 128) + j
                vps = mm_pool.tile([128, CH], F32, tag="mm", name="vps")

                def fv1(j=j, vps=vps):
                    for e in range(NE // 2):
                        nc.tensor.matmul(vps[:, 0:128], xts[e][:, ts(j, 128)],
                                         wv_sb[p][:, ts(e, 128)],
                                         start=(e == 0), stop=False)
                fillers.append((213, fv1))

                def fv2(j=j, tt=tt, vps=vps):
                    for e in range(NE // 2, NE):
                        nc.tensor.matmul(vps[:, 0:128], xts[e][:, ts(j, 128)],
                                         wv_sb[p][:, ts(e, 128)],
                                         start=False, stop=(e == NE - 1))
                    nc.vector.tensor_copy(
                        vsbs[p][:, tt, 0:130].rearrange(
                            "p (h c) -> p h c", h=2)[:, :, 0:64],
                        vps[:, 0:128].rearrange("p (h c) -> p h c", h=2))
                fillers.append((213, fv2))
            return fillers, qTt

        def outproj_fillers(qc, ctxns, lo=0, hi=CH // 128):
            fillers = []
            for j in range(lo, hi):
                tt = qc * (CH // 128) + j
                for eo in range(E // CH):
                    yp = mm_pool.tile([128, CH], F32, tag="mm", name="yp")

                    def f1(j=j, eo=eo, yp=yp):
                        for p in range(NP):
                            nc.tensor.matmul(yp[:], ctxns[p][:, ts(j, 128)],
                                             wo_sb[p][:, ts(eo, CH)],
                                             start=(p == 0), stop=(p == NP - 1))
                    fillers.append((426, f1))

                    def f2(tt=tt, eo=eo, yp=yp):
                        ysb = ysb_pool.tile([128, CH], dt_out, tag="ysb",
                                            name="ysb")
                        nc.vector.tensor_copy(ysb[:], yp[:])
                        if ablate != "noydma":
                            nc.sync.dma_start(
                                y[tt * 128:(tt + 1) * 128,
                                  eo * CH:(eo + 1) * CH], ysb[:])
                    fillers.append((0, f2))
            return fillers

        def attention(qc, p, qTt, fillers):
            """Attention over k-tiles, popping filler work between tiles to
            keep the PE busy through the ACT-bound exp stream."""
            nkt = (qc * CH + CH) // KT
            cps = [ctx_pool.tile([65, CH], F32, tag="ctx", name=f"ctx_ps{hi}")
                   for hi in range(2)]

            def emit_ctx(kt, ex, c0, n):
                for hi in range(2):
                    nc.tensor.matmul(
                        cps[hi][:, c0:CH],
                        vsbs[p][:, kt, hi * 65:(hi + 1) * 65],
                        ex[:, hi, 0:n],
                        start=(kt == 0), stop=(kt == nkt - 1),
                        skip_group_check=True,
                    )

            deficit = 1200.0  # warmup: first exp latency to hide
            filled = 0.0
            prev_ctx = None
            for kt in range(nkt):
                c0 = max(0, kt * KT - qc * CH)
                n = CH - c0
                diag = kt * KT >= qc * CH
                sc = sc_pool.tile([128, 2, CH], F32, tag="sc", name="sc_ps")
                for hi in range(2):
                    r0 = hi * 64
                    nc.tensor.matmul(
                        sc[:, hi, 0:n],
                        kTs[p][r0:r0 + 64, ts(kt, KT)],
                        qTt[r0:r0 + 64, c0:CH],
                        start=True, stop=not diag,
                        skip_group_check=diag,
                    )
                    if diag:
                        # add -30000 to k>q entries of the 128-col diagonal
                        # block; exp then underflows to 0
                        nc.tensor.matmul(
                            sc[:, hi, 0:128], tri_sb[:], id_sb[:],
                            start=False, stop=True,
                            skip_group_check=True,
                        )
                ex = ex_pool.tile([128, 2, CH], dt_i, tag="ex", name="ex")
                # timing-only ablations: tinyexp shrinks the ACT op; decouple
                # additionally feeds ctx from a const (no ACT->PE edge)
                nex = 8 if ablate in ("tinyexp", "decouple") else n
                nc.scalar.activation(
                    ex[:, :, 0:nex], sc[:, :, 0:nex],
                    mybir.ActivationFunctionType.Exp, scale=1.0 / np.sqrt(D))
                if ablate == "decouple":
                    ex = exc_sb
                # ACT-vs-PE deficit for this tile: exp+overheads minus the
                # scores/ctx matmul time; pop that much filler PE work
                deficit += (2 * n * 0.833 + 356) - (4 * n * 0.4167)
                while fillers and filled < deficit:
                    cost, f = fillers.pop(0)
                    f()
                    filled += cost
                # delay ctx by one k-tile: scores(kt+1) issues on the PE
                # before ctx(kt), hiding the exp latency
                if prev_ctx is not None:
                    emit_ctx(*prev_ctx)
                prev_ctx = (kt, ex, c0, n)
            emit_ctx(*prev_ctx)
            for _, f in fillers:  # drain leftovers
                f()
            ctxn = cn_pool.tile([128, CH], dt_i, tag="ctxn", name="ctxn")
            return cps, ctxn

        pairs = [(qc, p) for qc in range(ncb) for p in range(NP)]
        xts_by = {0: alloc_x(0)}
        first, qTt_next = proj_fillers(0, 0, xts_by[0])
        for _, f in first:
            f()
        pending = None       # ctxn tiles of prev chunk awaiting out-proj
        pending_norm = None  # (cps, ctxn) of prev pair awaiting normalize
        ctxns = []
        for idx, (qc, p) in enumerate(pairs):
            qTt = qTt_next
            fillers = []
            if pending_norm is not None:
                fillers += norm_fillers(*pending_norm)
            if (qc, p) != pairs[-1]:
                nqc, npp = pairs[idx + 1]
                if npp == 0:
                    xts_by[nqc] = alloc_x(nqc)
                    xts_by.pop(nqc - 1, None)
                nf, qTt_next = proj_fillers(nqc, npp, xts_by[nqc])
                fillers += nf
            if pending is not None:
                half = CH // 256  # 2: out-proj t-tiles split across both pairs
                if p == 0:
                    fillers += outproj_fillers(qc - 1, pending, 0, half)
                else:
                    fillers += outproj_fillers(qc - 1, pending, half, CH // 128)
                    pending = None
            cps, ctxn = attention(qc, p, qTt, fillers)
            pending_norm = (cps, ctxn)
            ctxns.append(ctxn)
            if p == NP - 1:
                pending = ctxns
                ctxns = []

        for _, f in norm_fillers(*pending_norm):
            f()
        pending_norm = None
        for _, f in outproj_fillers(ncb - 1, pending):
            f()
        pending = None

        if rep_cm is not None:
            rep_cm.__exit__(None, None, None)

    nc.compile()
    return nc


def _prep_inputs(x, Wq, Wk, Wv, Wo, dt_in=np.float32):
    """Host-side sharding: transpose x per batch, slice weights per core."""
    batch, seq, _ = x.shape
    xTs = [np.ascontiguousarray(x[b].T).astype(dt_in) for b in range(batch)]
    # trineg[q, k] = -30000 iff k > q; used as lhsT with rhs=I so the scores
    # PSUM group accumulates -30000 into causally-invalid diagonal entries
    trineg = -30000.0 * np.triu(np.ones((128, 128), np.float32), 1)
    identity = np.eye(128, dtype=np.float32)

    def warr(w):  # [E, 128] col-slice -> SBUF layout [128, 8*128]
        return np.ascontiguousarray(
            w.reshape(NE, 128, 128).transpose(1, 0, 2).reshape(128, E)
        ).astype(dt_in)

    in_maps = []
    for i in range(N_CORES):
        b, g = i // NG, i % NG
        cs = [slice(g * 256 + p * 128, g * 256 + (p + 1) * 128)
              for p in range(NP)]
        in_maps.append({
            "xT": xTs[b],
            "wq": np.concatenate([warr(Wq[:, c]) for c in cs], axis=0),
            "wk": np.concatenate([warr(Wk[:, c]) for c in cs], axis=0),
            "wv": np.concatenate([warr(Wv[:, c]) for c in cs], axis=0),
            "wo": np.concatenate(
                [np.ascontiguousarray(Wo[c, :]) for c in cs], axis=0
            ).astype(dt_in),
            "trineg": trineg.astype(dt_in),
            "iden": identity.astype(dt_in),
            "onesc": np.ones((128, 1), dt_in),
        })
    return in_maps


_CACHE = {}


def _get_nc(batch, seq, dt_in, dt_out):
    key = (batch, seq, dt_in, dt_out)
    if key not in _CACHE:
        _CACHE[key] = build_attention(batch, seq, dt_in, dt_out)
    return _CACHE[key]


DT_IN = F16   # fp16 x/W transfers; projections accumulate fp32 in PSUM
DT_OUT = F16  # fp16 partial-y transfers; host sums in fp32


def kernel(x, Wq, Wk, Wv, Wo, bo, _trace=False):
    x = np.asarray(x, np.float32)
    batch, seq, _ = x.shape
    nc = _get_nc(batch, seq, DT_IN, DT_OUT)
    in_maps = _prep_inputs(x, np.asarray(Wq), np.asarray(Wk), np.asarray(Wv),
                           np.asarray(Wo),
                           dt_in=np.float16 if DT_IN == F16 else np.float32)
    res = run_bass_kernel_spmd(nc, in_maps, core_ids=list(range(N_CORES)),
                               trace=_trace)
    y = np.empty((batch, seq, E), np.float32)
    bo32 = np.asarray(bo, np.float32)
    for b in range(NB):
        parts = [res.results[b * NG + g]["y"].astype(np.float32)
                 for g in range(NG)]
        y[b] = np.sum(parts, axis=0, dtype=np.float32) + bo32
    if _trace:
        kernel.last_results = res
    return y


# revision 26
# speedup vs baseline: 1.2611x; 1.2429x over previous
"""Causal self-attention kernel for Trainium2, sharded over 8 NeuronCores.

Problem (hardcoded): x [2, 2048, 1024] fp32, Wq/Wk/Wv/Wo [1024, 1024], bo [1024].
H = 16 heads, head dim 64.

Sharding: batch x head-group hybrid. Core i handles batch i//4 and head group
i%4 (4 heads = 256 features, processed as 2 pairs of 2 heads). Each core
computes its partial out-projection y_i = ctx_i @ Wo[rows_i] for its batch;
the host sums the 4 partials per batch (the "all-reduce") and adds bo.
vs pure head-parallel TP this halves per-core x-in / y-out DMA and the
PSUM->SBUF output copy traffic.

On-device layout (per core, T = 2048 tokens of its batch):

  x           loaded chunk-wise ([128, 512] e-tiles, double-buffered) as xT
  qT, kT      [128 (2 heads x 64 d), cols] feature-major from W.T-slices as
              lhsT, xT as rhs; q is per-chunk, k accumulates over the batch
  v           token-major DIRECTLY from projection (lhsT = xT tile, rhs = Wv
              slice) -> [t, d]; no PE transposes. Stored as [k, (vA|1|vB|1)]
              so ctx row 64 accumulates the softmax denominator for free
  scoresT     [k-tile 128, 2 heads, q-chunk 512] in a 2-bank PSUM tile
  expT        exp(scoresT / 8) for both heads in ONE ScalarE activation;
              causal-masked on DVE via a triangular 0/1 mask
  ctxT (+l)   [65, 512] per head; normalized via reciprocal (DVE) +
              partition_broadcast (GPSIMD) + mul (DVE)
  y partial   lhsT = normalized ctxT t-tile per pair, rhs = Wo local rows,
              2 accumulating matmuls (one per pair). Out-projection of chunk
              qc is emitted after the projections of chunk qc+1 so the
              normalize latency never stalls the PE queue.

PSUM (8 banks): sc 2x2 + ctx 2x1 + mm (psq/psk/vps/yp ring) 2x1.
"""

import sys

import numpy as np

try:
    import concourse.bass as bass  # noqa: F401
except ImportError:  # harness environments without concourse on sys.path
    sys.path.insert(0, "/opt/trn_rl_repo")
    import concourse.bass as bass  # noqa: F401

from contextlib import ExitStack

import concourse.mybir as mybir
import concourse.tile as tile
from concourse import bacc
from concourse.bass import ts
from concourse.bass_utils import run_bass_kernel_spmd

F32 = mybir.dt.float32
F32R = mybir.dt.float32r
F16 = mybir.dt.float16

N_CORES = 8
B, S, E = 2, 2048, 1024
H, D = 16, 64
NB = 2            # batch shards
NG = 4            # head-group shards (4 heads each)
NP = 2            # head pairs per core
EL = 128          # features per pair (2 heads x 64)
CH = 512          # q-chunk width (one PSUM bank of fp32)
KT = 128          # k-tile width
NE = E // 128     # e-tiles in the contraction dim


def build_attention(batch=B, seq=S, dt_in=F32, dt_out=F32, n_reps=1):
    """Build the per-core Bass program (same program on all 8 cores)."""
    import os
    ablate = os.environ.get("ABLATE", "none")  # timing experiments only
    ncb = seq // CH            # q-chunks (per-core batch)
    nkt_b = seq // KT          # k-tiles

    nc = bacc.Bacc("TRN2", debug=False, num_devices=N_CORES)

    dt_ind = F32R if dt_in == F32 else dt_in
    dt_i = dt_ind
    xT = nc.dram_tensor("xT", [E, seq], dt_ind, kind="ExternalInput").ap()
    wq = nc.dram_tensor("wq", [NP * 128, E], dt_ind, kind="ExternalInput").ap()
    wk = nc.dram_tensor("wk", [NP * 128, E], dt_ind, kind="ExternalInput").ap()
    wv = nc.dram_tensor("wv", [NP * 128, E], dt_ind, kind="ExternalInput").ap()
    wo = nc.dram_tensor("wo", [NP * 128, E], dt_ind, kind="ExternalInput").ap()
    # trineg[q, k] = -30000 iff k > q (strict upper): rank-128 causal mask
    # added into the scores PSUM group via lhsT=trineg, rhs=iden
    trineg = nc.dram_tensor("trineg", [128, 128], dt_ind, kind="ExternalInput").ap()
    iden = nc.dram_tensor("iden", [128, 128], dt_ind, kind="ExternalInput").ap()
    onesc = nc.dram_tensor("onesc", [128, 1], dt_ind, kind="ExternalInput").ap()
    y = nc.dram_tensor("y", [seq, E], dt_out, kind="ExternalOutput").ap()

    with tile.TileContext(nc) as tc, ExitStack() as ctx, \
            nc.allow_low_precision(reason="fp16 internals validated vs reference"):
        consts = ctx.enter_context(tc.tile_pool(name="consts", bufs=1))
        xt_pool = ctx.enter_context(tc.tile_pool(name="xt", bufs=2 * NE))
        big = ctx.enter_context(tc.tile_pool(name="big", bufs=1))
        q_pool = ctx.enter_context(tc.tile_pool(name="qp", bufs=2))
        ex_pool = ctx.enter_context(tc.tile_pool(name="ex", bufs=6))
        cn_pool = ctx.enter_context(tc.tile_pool(name="cn", bufs=4))
        ysb_pool = ctx.enter_context(tc.tile_pool(name="ysb", bufs=4))
        small = ctx.enter_context(tc.tile_pool(name="small", bufs=3))
        # PSUM (8 banks): sc 2x2 + ctx 2x1 + mm 2x1
        sc_pool = ctx.enter_context(tc.tile_pool(name="scps", bufs=2, space="PSUM"))
        ctx_pool = ctx.enter_context(tc.tile_pool(name="ctxps", bufs=2, space="PSUM"))
        mm_pool = ctx.enter_context(tc.tile_pool(name="mmps", bufs=2, space="PSUM"))

        # ---- constants / weights (resident, loaded once per dispatch) ----
        wq_sb, wk_sb, wv_sb, wo_sb = [], [], [], []
        for p in range(NP):
            for lst, src, nm in ((wq_sb, wq, "wq"), (wk_sb, wk, "wk"),
                                 (wv_sb, wv, "wv"), (wo_sb, wo, "wo")):
                t = consts.tile([128, E], dt_ind, name=f"{nm}{p}_sb")
                nc.sync.dma_start(t[:], src[p * 128:(p + 1) * 128, :])
                lst.append(t)
        tri_sb = consts.tile([128, 128], dt_ind, name="tri_sb")
        nc.sync.dma_start(tri_sb[:], trineg)
        id_sb = consts.tile([128, 128], dt_ind, name="id_sb")
        nc.sync.dma_start(id_sb[:], iden)
        ones_sb = consts.tile([128, 1], dt_ind, name="ones_sb")
        nc.sync.dma_start(ones_sb[:], onesc)
        if ablate == "decouple":
            exc_sb = consts.tile([128, 2, CH], dt_i, name="exc_sb")
            nc.vector.memset(exc_sb[:], 1.0)

        rep_cm = tc.For_i(0, n_reps, 1) if n_reps > 1 else None
        if rep_cm is not None:
            rep_cm.__enter__()

        # persistent per-pair K / V state for the whole batch
        kTs, vsbs = [], []
        for p in range(NP):
            kT = big.tile([128, seq], dt_i, tag=f"kT{p}", name=f"kT{p}", bufs=1)
            vsb = big.tile([128, nkt_b, 130], dt_i, tag=f"vsb{p}",
                           name=f"vsb{p}", bufs=1)
            nc.vector.tensor_copy(
                vsb.rearrange("p t (h c) -> p (t h) c", c=65)[:, :, 64:65],
                ones_sb.broadcast_to([128, 2 * nkt_b, 1]))
            kTs.append(kT)
            vsbs.append(vsb)

        def alloc_x(qc):
            xts = []
            for e in range(NE):
                xt = xt_pool.tile([128, CH], dt_ind, tag="xt", name="xt")
                nc.sync.dma_start(xt[:], xT[e * 128:(e + 1) * 128,
                                            qc * CH:(qc + 1) * CH])
                xts.append(xt)
            return xts

        def norm_fillers(cps, ctxn):
            """Normalize ctxT[0:64] / l (row 64) for both heads."""
            def f():
                for hi in range(2):
                    rc = small.tile([1, CH], F32, tag="rc", name="rc")
                    nc.vector.reciprocal(rc[:], cps[hi][64:65, :])
                    bc = small.tile([64, CH], F32, tag="bc", name="bc")
                    nc.gpsimd.partition_broadcast(bc[:], rc[:])
                    nc.vector.tensor_mul(
                        ctxn[hi * 64:(hi + 1) * 64, :], cps[hi][0:64, :], bc[:])
            return [(0, f)]

        def proj_fillers(qc, p, xts):
            """q/k/v projection of (qc, p) as (cost_ns, closure) filler units.
            Tiles are allocated now (ring order = FIFO pop order)."""
            psq = mm_pool.tile([128, CH], F32, tag="mm", name="psq")
            psk = mm_pool.tile([128, CH], F32, tag="mm", name="psk")
            qTt = q_pool.tile([128, CH], dt_i, tag="qT", name="qTt")
            fillers = []
            for ps, w_sb, done in ((psq, wq_sb[p], None), (psk, wk_sb[p], None)):
                for e0 in range(0, NE, 2):
                    def f(ps=ps, w_sb=w_sb, e0=e0):
                        for e in (e0, e0 + 1):
                            nc.tensor.matmul(ps[:], w_sb[:, ts(e, 128)],
                                             xts[e][:],
                                             start=(e == 0), stop=(e == NE - 1))
                    fillers.append((426, f))

            def fq():
                nc.vector.tensor_copy(qTt[:], psq[:])
            fillers.append((0, fq))

            def fk():
                nc.vector.tensor_copy(kTs[p][:, ts(qc, CH)], psk[:])
            fillers.append((0, fk))

            for j in range(CH // 128):
                tt = qc * (CH // 128) + j
                vps = mm_pool.tile([128, CH], F32, tag="mm", name="vps")

                def fv1(j=j, vps=vps):
                    for e in range(NE // 2):
                        nc.tensor.matmul(vps[:, 0:128], xts[e][:, ts(j, 128)],
                                         wv_sb[p][:, ts(e, 128)],
                                         start=(e == 0), stop=False)
                fillers.append((213, fv1))

                def fv2(j=j, tt=tt, vps=vps):
                    for e in range(NE // 2, NE):
                        nc.tensor.matmul(vps[:, 0:128], xts[e][:, ts(j, 128)],
                                         wv_sb[p][:, ts(e, 128)],
                                         start=False, stop=(e == NE - 1))
                    nc.vector.tensor_copy(
                        vsbs[p][:, tt, 0:130].rearrange(
                            "p (h c) -> p h c", h=2)[:, :, 0:64],
                        vps[:, 0:128].rearrange("p (h c) -> p h c", h=2))
                fillers.append((213, fv2))
            return fillers, qTt

        def outproj_fillers(qc, ctxns, lo=0, hi=CH // 128):
            fillers = []
            for j in range(lo, hi):
                tt = qc * (CH // 128) + j
                for eo in range(E // CH):
                    yp = mm_pool.tile([128, CH], F32, tag="mm", name="yp")

                    def f1(j=j, eo=eo, yp=yp):
                        for p in range(NP):
                            nc.tensor.matmul(yp[:], ctxns[p][:, ts(j, 128)],
                                             wo_sb[p][:, ts(eo, CH)],
                                             start=(p == 0), stop=(p == NP - 1))
                    fillers.append((426, f1))

                    def f2(tt=tt, eo=eo, yp=yp):
                        ysb = ysb_pool.tile([128, CH], dt_out, tag="ysb",
                                            name="ysb")
                        nc.vector.tensor_copy(ysb[:], yp[:])
                        if ablate != "noydma":
                            nc.sync.dma_start(
                                y[tt * 128:(tt + 1) * 128,
                                  eo * CH:(eo + 1) * CH], ysb[:])
                    fillers.append((0, f2))
            return fillers

        def attention(qc, p, qTt, fillers):
            """Attention over k-tiles, popping filler work between tiles to
            keep the PE busy through the ACT-bound exp stream."""
            nkt = (qc * CH + CH) // KT
            cps = [ctx_pool.tile([65, CH], F32, tag="ctx", name=f"ctx_ps{hi}")
                   for hi in range(2)]

            def emit_ctx(kt, ex, c0, n):
                for hi in range(2):
                    nc.tensor.matmul(
                        cps[hi][:, c0:CH],
                        vsbs[p][:, kt, hi * 65:(hi + 1) * 65],
                        ex[:, hi, 0:n],
                        start=(kt == 0), stop=(kt == nkt - 1),
                        skip_group_check=True,
                    )

            deficit = 1200.0  # warmup: first exp latency to hide
            filled = 0.0
            prev_ctx = None
            for kt in range(nkt):
                c0 = max(0, kt * KT - qc * CH)
                n = CH - c0
                diag = kt * KT >= qc * CH
                sc = sc_pool.tile([128, 2, CH], F32, tag="sc", name="sc_ps")
                for hi in range(2):
                    r0 = hi * 64
                    nc.tensor.matmul(
                        sc[:, hi, 0:n],
                        kTs[p][r0:r0 + 64, ts(kt, KT)],
                        qTt[r0:r0 + 64, c0:CH],
                        start=True, stop=not diag,
                        skip_group_check=diag,
                    )
                    if diag:
                        # add -30000 to k>q entries of the 128-col diagonal
                        # block; exp then underflows to 0
                        nc.tensor.matmul(
                            sc[:, hi, 0:128], tri_sb[:], id_sb[:],
                            start=False, stop=True,
                            skip_group_check=True,
                        )
                ex = ex_pool.tile([128, 2, CH], dt_i, tag="ex", name="ex")
                # timing-only ablations: tinyexp shrinks the ACT op; decouple
                # additionally feeds ctx from a const (no ACT->PE edge)
                nex = 8 if ablate in ("tinyexp", "decouple") else n
                nc.scalar.activation(
                    ex[:, :, 0:nex], sc[:, :, 0:nex],
                    mybir.ActivationFunctionType.Exp, scale=1.0 / np.sqrt(D))
                if ablate == "decouple":
                    ex = exc_sb
                # ACT-vs-PE deficit for this tile: exp+overheads minus the
                # scores/ctx matmul time; pop that much filler PE work
                deficit += (2 * n * 0.833 + 356) - (4 * n * 0.4167)
                while fillers and filled < deficit:
                    cost, f = fillers.pop(0)
                    f()
                    filled += cost
                # delay ctx by one k-tile: scores(kt+1) issues on the PE
                # before ctx(kt), hiding the exp latency
                if prev_ctx is not None:
                    emit_ctx(*prev_ctx)
                prev_ctx = (kt, ex, c0, n)
            emit_ctx(*prev_ctx)
            for _, f in fillers:  # drain leftovers
                f()
            ctxn = cn_pool.tile([128, CH], dt_i, tag="ctxn", name="ctxn")
            return cps, ctxn

        if ablate == "projonly":
            # timing skeleton: projections + out-proj only, no attention
            for qc in range(ncb):
                xts = alloc_x(qc)
                ctxns = []
                for p in range(NP):
                    fl, _ = proj_fillers(qc, p, xts)
                    for _, f in fl:
                        f()
                    ctxn = cn_pool.tile([128, CH], dt_i, tag="ctxn",
                                        name="ctxn")
                    nc.vector.memset(ctxn[:], 0.5)
                    ctxns.append(ctxn)
                for _, f in outproj_fillers(qc, ctxns):
                    f()
        elif ablate == "attnonly":
            # timing skeleton: attention only; q/k/v from consts, no proj/IO
            kc = consts.tile([128, seq], dt_i, name="kc")
            nc.vector.memset(kc[:], 0.01)
            vc = consts.tile([128, nkt_b, 130], dt_i, name="vc")
            nc.vector.memset(vc[:], 0.01)
            qc_t = consts.tile([128, CH], dt_i, name="qc_t")
            nc.vector.memset(qc_t[:], 0.01)
            kTs[:] = [kc] * NP
            vsbs[:] = [vc] * NP
            for qc in range(ncb):
                for p in range(NP):
                    cps, ctxn = attention(qc, p, qc_t, [])
                    for _, f in norm_fillers(cps, ctxn):
                        f()
        else:
            pairs = [(qc, p) for qc in range(ncb) for p in range(NP)]
            xts_by = {0: alloc_x(0)}
            first, qTt_next = proj_fillers(0, 0, xts_by[0])
            for _, f in first:
                f()
            pending = None       # ctxn tiles of prev chunk awaiting out-proj
            pending_norm = None  # (cps, ctxn) of prev pair, not normalized
            ctxns = []
            for idx, (qc, p) in enumerate(pairs):
                qTt = qTt_next
                fillers = []
                if pending_norm is not None:
                    fillers += norm_fillers(*pending_norm)
                if (qc, p) != pairs[-1]:
                    nqc, npp = pairs[idx + 1]
                    if npp == 0:
                        xts_by[nqc] = alloc_x(nqc)
                        xts_by.pop(nqc - 1, None)
                    nf, qTt_next = proj_fillers(nqc, npp, xts_by[nqc])
                    fillers += nf
                if pending is not None:
                    half = CH // 256  # out-proj t-tiles split across pairs
                    if p == 0:
                        fillers += outproj_fillers(qc - 1, pending, 0, half)
                    else:
                        fillers += outproj_fillers(qc - 1, pending, half,
                                                   CH // 128)
                        pending = None
                cps, ctxn = attention(qc, p, qTt, fillers)
                pending_norm = (cps, ctxn)
                ctxns.append(ctxn)
                if p == NP - 1:
                    pending = ctxns
                    ctxns = []

            for _, f in norm_fillers(*pending_norm):
                f()
            pending_norm = None
            for _, f in outproj_fillers(ncb - 1, pending):
                f()
            pending = None

        if rep_cm is not None:
            rep_cm.__exit__(None, None, None)

    nc.compile()
    return nc


def _prep_inputs(x, Wq, Wk, Wv, Wo, dt_in=np.float32):
    """Host-side sharding: transpose x per batch, slice weights per core."""
    batch, seq, _ = x.shape
    xTs = [np.ascontiguousarray(x[b].T).astype(dt_in) for b in range(batch)]
    # trineg[q, k] = -30000 iff k > q; used as lhsT with rhs=I so the scores
    # PSUM group accumulates -30000 into causally-invalid diagonal entries
    trineg = -30000.0 * np.triu(np.ones((128, 128), np.float32), 1)
    identity = np.eye(128, dtype=np.float32)

    def warr(w):  # [E, 128] col-slice -> SBUF layout [128, 8*128]
        return np.ascontiguousarray(
            w.reshape(NE, 128, 128).transpose(1, 0, 2).reshape(128, E)
        ).astype(dt_in)

    in_maps = []
    for i in range(N_CORES):
        b, g = i // NG, i % NG
        cs = [slice(g * 256 + p * 128, g * 256 + (p + 1) * 128)
              for p in range(NP)]
        in_maps.append({
            "xT": xTs[b],
            "wq": np.concatenate([warr(Wq[:, c]) for c in cs], axis=0),
            "wk": np.concatenate([warr(Wk[:, c]) for c in cs], axis=0),
            "wv": np.concatenate([warr(Wv[:, c]) for c in cs], axis=0),
            "wo": np.concatenate(
                [np.ascontiguousarray(Wo[c, :]) for c in cs], axis=0
            ).astype(dt_in),
            "trineg": trineg.astype(dt_in),
            "iden": identity.astype(dt_in),
            "onesc": np.ones((128, 1), dt_in),
        })
    return in_maps


_CACHE = {}


def _get_nc(batch, seq, dt_in, dt_out):
    key = (batch, seq, dt_in, dt_out)
    if key not in _CACHE:
        _CACHE[key] = build_attention(batch, seq, dt_in, dt_out)
    return _CACHE[key]


DT_IN = F16   # fp16 x/W transfers; projections accumulate fp32 in PSUM
DT_OUT = F16  # fp16 partial-y transfers; host sums in fp32


def kernel(x, Wq, Wk, Wv, Wo, bo, _trace=False):
    x = np.asarray(x, np.float32)
    batch, seq, _ = x.shape
    nc = _get_nc(batch, seq, DT_IN, DT_OUT)
    in_maps = _prep_inputs(x, np.asarray(Wq), np.asarray(Wk), np.asarray(Wv),
                           np.asarray(Wo),
                           dt_in=np.float16 if DT_IN == F16 else np.float32)
    res = run_bass_kernel_spmd(nc, in_maps, core_ids=list(range(N_CORES)),
                               trace=_trace)
    y = np.empty((batch, seq, E), np.float32)
    bo32 = np.asarray(bo, np.float32)
    for b in range(NB):
        parts = [res.results[b * NG + g]["y"].astype(np.float32)
                 for g in range(NG)]
        y[b] = np.sum(parts, axis=0, dtype=np.float32) + bo32
    if _trace:
        kernel.last_results = res
    return y


# revision 29
# speedup vs baseline: 1.5129x; 1.1997x over previous
"""Causal self-attention kernel for Trainium2, sharded over 8 NeuronCores.

Problem (hardcoded): x [2, 2048, 1024] fp32, Wq/Wk/Wv/Wo [1024, 1024], bo [1024].
H = 16 heads, head dim 64.

Sharding: batch x head-group hybrid. Core i handles batch i//4 and head group
i%4 (4 heads = 256 features, processed as 2 pairs of 2 heads). Each core
computes its partial out-projection y_i = ctx_i @ Wo[rows_i] for its batch;
the host sums the 4 partials per batch (the "all-reduce") and adds bo.
vs pure head-parallel TP this halves per-core x-in / y-out DMA and the
PSUM->SBUF output copy traffic.

On-device layout (per core, T = 2048 tokens of its batch):

  x           loaded chunk-wise ([128, 512] e-tiles, double-buffered) as xT
  qT, kT      [128 (2 heads x 64 d), cols] feature-major from W.T-slices as
              lhsT, xT as rhs; q is per-chunk, k accumulates over the batch
  v           token-major DIRECTLY from projection (lhsT = xT tile, rhs = Wv
              slice) -> [t, d]; no PE transposes. Stored as [k, (vA|1|vB|1)]
              so ctx row 64 accumulates the softmax denominator for free
  scoresT     [k-tile 128, 2 heads, q-chunk 512] in a 2-bank PSUM tile
  expT        exp(scoresT / 8) for both heads in ONE ScalarE activation;
              causal-masked on DVE via a triangular 0/1 mask
  ctxT (+l)   [65, 512] per head; normalized via reciprocal (DVE) +
              partition_broadcast (GPSIMD) + mul (DVE)
  y partial   lhsT = normalized ctxT t-tile per pair, rhs = Wo local rows,
              2 accumulating matmuls (one per pair). Out-projection of chunk
              qc is emitted after the projections of chunk qc+1 so the
              normalize latency never stalls the PE queue.

PSUM (8 banks): sc 2x2 + ctx 2x1 + mm (psq/psk/vps/yp ring) 2x1.
"""

import sys

import numpy as np

try:
    import concourse.bass as bass  # noqa: F401
except ImportError:  # harness environments without concourse on sys.path
    sys.path.insert(0, "/opt/trn_rl_repo")
    import concourse.bass as bass  # noqa: F401

from contextlib import ExitStack

import concourse.mybir as mybir
import concourse.tile as tile
from concourse import bacc
from concourse.bass import ts
from concourse.bass_utils import run_bass_kernel_spmd

F32 = mybir.dt.float32
F32R = mybir.dt.float32r
F16 = mybir.dt.float16

N_CORES = 8
B, S, E = 2, 2048, 1024
H, D = 16, 64
NB = 2            # batch shards
NG = 4            # head-group shards (4 heads each)
NP = 2            # head pairs per core
EL = 128          # features per pair (2 heads x 64)
CH = 512          # q-chunk width (one PSUM bank of fp32)
KT = 128          # k-tile width
NE = E // 128     # e-tiles in the contraction dim


def build_attention(batch=B, seq=S, dt_in=F32, dt_out=F32, n_reps=1):
    """Build the per-core Bass program (same program on all 8 cores)."""
    import os
    ablate = os.environ.get("ABLATE", "none")  # timing experiments only
    sc_bufs = int(os.environ.get("SCBUFS", "2"))
    ctx_delay = int(os.environ.get("CTXDELAY", "1"))
    mask_dve = os.environ.get("MASKDVE", "0") == "1"
    ncb = seq // CH            # q-chunks (per-core batch)
    nkt_b = seq // KT          # k-tiles

    nc = bacc.Bacc("TRN2", debug=False, num_devices=N_CORES)

    dt_ind = F32R if dt_in == F32 else dt_in
    dt_i = dt_ind
    xT = nc.dram_tensor("xT", [E, seq], dt_ind, kind="ExternalInput").ap()
    wq = nc.dram_tensor("wq", [NP * 128, E], dt_ind, kind="ExternalInput").ap()
    wk = nc.dram_tensor("wk", [NP * 128, E], dt_ind, kind="ExternalInput").ap()
    wv = nc.dram_tensor("wv", [NP * 128, E], dt_ind, kind="ExternalInput").ap()
    wo = nc.dram_tensor("wo", [NP * 128, E], dt_ind, kind="ExternalInput").ap()
    # trineg[q, k] = -30000 iff k > q (strict upper): rank-128 causal mask
    # added into the scores PSUM group via lhsT=trineg, rhs=iden
    trineg = nc.dram_tensor("trineg", [128, 128], dt_ind, kind="ExternalInput").ap()
    iden = nc.dram_tensor("iden", [128, 128], dt_ind, kind="ExternalInput").ap()
    trimask = nc.dram_tensor("trimask", [128, 128], dt_ind, kind="ExternalInput").ap()
    onesc = nc.dram_tensor("onesc", [128, 1], dt_ind, kind="ExternalInput").ap()
    y = nc.dram_tensor("y", [seq, E], dt_out, kind="ExternalOutput").ap()

    with tile.TileContext(nc) as tc, ExitStack() as ctx, \
            nc.allow_low_precision(reason="fp16 internals validated vs reference"):
        consts = ctx.enter_context(tc.tile_pool(name="consts", bufs=1))
        xt_pool = ctx.enter_context(tc.tile_pool(name="xt", bufs=2 * NE))
        big = ctx.enter_context(tc.tile_pool(name="big", bufs=1))
        q_pool = ctx.enter_context(tc.tile_pool(name="qp", bufs=2))
        ex_pool = ctx.enter_context(tc.tile_pool(name="ex", bufs=6))
        cn_pool = ctx.enter_context(tc.tile_pool(name="cn", bufs=4))
        ysb_pool = ctx.enter_context(tc.tile_pool(name="ysb", bufs=4))
        small = ctx.enter_context(tc.tile_pool(name="small", bufs=3))
        # PSUM (8 banks): sc 2x2 + ctx 2x1 + mm 2x1
        sc_pool = ctx.enter_context(tc.tile_pool(name="scps", bufs=2, space="PSUM"))
        ctx_pool = ctx.enter_context(tc.tile_pool(name="ctxps", bufs=2, space="PSUM"))
        mm_pool = ctx.enter_context(tc.tile_pool(name="mmps", bufs=2, space="PSUM"))

        # ---- constants / weights (resident, loaded once per dispatch) ----
        wq_sb, wk_sb, wv_sb, wo_sb = [], [], [], []
        for p in range(NP):
            for lst, src, nm in ((wq_sb, wq, "wq"), (wk_sb, wk, "wk"),
                                 (wv_sb, wv, "wv"), (wo_sb, wo, "wo")):
                t = consts.tile([128, E], dt_ind, name=f"{nm}{p}_sb")
                nc.sync.dma_start(t[:], src[p * 128:(p + 1) * 128, :])
                lst.append(t)
        tri_sb = consts.tile([128, 128], dt_ind, name="tri_sb")
        nc.sync.dma_start(tri_sb[:], trineg)
        id_sb = consts.tile([128, 128], dt_ind, name="id_sb")
        nc.sync.dma_start(id_sb[:], iden)
        trim_sb = consts.tile([128, 128], dt_ind, name="trim_sb")
        nc.sync.dma_start(trim_sb[:], trimask)
        ones_sb = consts.tile([128, 1], dt_ind, name="ones_sb")
        nc.sync.dma_start(ones_sb[:], onesc)
        if ablate == "decouple":
            exc_sb = consts.tile([128, 2, CH], dt_i, name="exc_sb")
            nc.vector.memset(exc_sb[:], 1.0)

        rep_cm = tc.For_i(0, n_reps, 1) if n_reps > 1 else None
        if rep_cm is not None:
            rep_cm.__enter__()

        # persistent per-pair K / V state for the whole batch
        kTs, vsbs = [], []
        for p in range(NP):
            kT = big.tile([128, seq], dt_i, tag=f"kT{p}", name=f"kT{p}", bufs=1)
            vsb = big.tile([128, nkt_b, 130], dt_i, tag=f"vsb{p}",
                           name=f"vsb{p}", bufs=1)
            nc.vector.tensor_copy(
                vsb.rearrange("p t (h c) -> p (t h) c", c=65)[:, :, 64:65],
                ones_sb.broadcast_to([128, 2 * nkt_b, 1]))
            kTs.append(kT)
            vsbs.append(vsb)

        def alloc_x(qc):
            xts = []
            for e in range(NE):
                xt = xt_pool.tile([128, CH], dt_ind, tag="xt", name="xt")
                nc.sync.dma_start(xt[:], xT[e * 128:(e + 1) * 128,
                                            qc * CH:(qc + 1) * CH])
                xts.append(xt)
            return xts

        def norm_fillers(cps, ctxn):
            """Normalize ctxT[0:64] / l (row 64) for both heads."""
            def f():
                for hi in range(2):
                    rc = small.tile([1, CH], F32, tag="rc", name="rc")
                    nc.vector.reciprocal(rc[:], cps[hi][64:65, :])
                    bc = small.tile([64, CH], F32, tag="bc", name="bc")
                    nc.gpsimd.partition_broadcast(bc[:], rc[:])
                    nc.vector.tensor_mul(
                        ctxn[hi * 64:(hi + 1) * 64, :], cps[hi][0:64, :], bc[:])
            return [(0, f)]

        def proj_fillers(qc, p, xts):
            """q/k/v projection of (qc, p) as (cost_ns, closure) filler units.
            Tiles are allocated now (ring order = FIFO pop order)."""
            psq = mm_pool.tile([128, CH], F32, tag="mm", name="psq")
            psk = mm_pool.tile([128, CH], F32, tag="mm", name="psk")
            qTt = q_pool.tile([128, CH], dt_i, tag="qT", name="qTt")
            fillers = []
            for ps, w_sb, done in ((psq, wq_sb[p], None), (psk, wk_sb[p], None)):
                for e0 in range(0, NE, 2):
                    def f(ps=ps, w_sb=w_sb, e0=e0):
                        for e in (e0, e0 + 1):
                            nc.tensor.matmul(ps[:], w_sb[:, ts(e, 128)],
                                             xts[e][:],
                                             start=(e == 0), stop=(e == NE - 1))
                    fillers.append((426, f))

            def fq():
                nc.vector.tensor_copy(qTt[:], psq[:])
            fillers.append((0, fq))

            def fk():
                nc.vector.tensor_copy(kTs[p][:, ts(qc, CH)], psk[:])
            fillers.append((0, fk))

            for j in range(CH // 128):
                tt = qc * (CH // 128) + j
                vps = mm_pool.tile([128, CH], F32, tag="mm", name="vps")

                def fv1(j=j, vps=vps):
                    for e in range(NE // 2):
                        nc.tensor.matmul(vps[:, 0:128], xts[e][:, ts(j, 128)],
                                         wv_sb[p][:, ts(e, 128)],
                                         start=(e == 0), stop=False)
                fillers.append((213, fv1))

                def fv2(j=j, tt=tt, vps=vps):
                    for e in range(NE // 2, NE):
                        nc.tensor.matmul(vps[:, 0:128], xts[e][:, ts(j, 128)],
                                         wv_sb[p][:, ts(e, 128)],
                                         start=False, stop=(e == NE - 1))
                    nc.vector.tensor_copy(
                        vsbs[p][:, tt, 0:130].rearrange(
                            "p (h c) -> p h c", h=2)[:, :, 0:64],
                        vps[:, 0:128].rearrange("p (h c) -> p h c", h=2))
                fillers.append((213, fv2))
            return fillers, qTt

        def outproj_fillers(qc, ctxns, lo=0, hi=CH // 128):
            fillers = []
            for j in range(lo, hi):
                tt = qc * (CH // 128) + j
                for eo in range(E // CH):
                    yp = mm_pool.tile([128, CH], F32, tag="mm", name="yp")

                    def f1(j=j, eo=eo, yp=yp):
                        for p in range(NP):
                            nc.tensor.matmul(yp[:], ctxns[p][:, ts(j, 128)],
                                             wo_sb[p][:, ts(eo, CH)],
                                             start=(p == 0), stop=(p == NP - 1))
                    fillers.append((426, f1))

                    def f2(tt=tt, eo=eo, yp=yp):
                        ysb = ysb_pool.tile([128, CH], dt_out, tag="ysb",
                                            name="ysb")
                        nc.vector.tensor_copy(ysb[:], yp[:])
                        if ablate != "noydma":
                            nc.sync.dma_start(
                                y[tt * 128:(tt + 1) * 128,
                                  eo * CH:(eo + 1) * CH], ysb[:])
                    fillers.append((0, f2))
            return fillers

        def attention(qc, p, qTt, fillers):
            """Attention over k-tiles, popping filler work between tiles to
            keep the PE busy through the ACT-bound exp stream."""
            nkt = (qc * CH + CH) // KT
            cps = [ctx_pool.tile([65, CH], F32, tag="ctx", name=f"ctx_ps{hi}")
                   for hi in range(2)]

            def emit_ctx(kt, ex, c0, n):
                for hi in range(2):
                    nc.tensor.matmul(
                        cps[hi][:, c0:CH],
                        vsbs[p][:, kt, hi * 65:(hi + 1) * 65],
                        ex[:, hi, 0:n],
                        start=(kt == 0), stop=(kt == nkt - 1),
                        skip_group_check=True,
                    )

            deficit = 1200.0  # warmup: first exp latency to hide
            filled = 0.0
            ctx_q = []
            for kt in range(nkt):
                c0 = max(0, kt * KT - qc * CH)
                n = CH - c0
                diag = kt * KT >= qc * CH
                sc = sc_pool.tile([128, 2, CH], F32, tag="sc", name="sc_ps",
                                  bufs=sc_bufs)
                for hi in range(2):
                    r0 = hi * 64
                    hard_mask = diag and not mask_dve
                    nc.tensor.matmul(
                        sc[:, hi, 0:n],
                        kTs[p][r0:r0 + 64, ts(kt, KT)],
                        qTt[r0:r0 + 64, c0:CH],
                        start=True, stop=not hard_mask,
                        skip_group_check=hard_mask,
                    )
                    if hard_mask:
                        # add -30000 to k>q entries of the 128-col diagonal
                        # block; exp then underflows to 0
                        nc.tensor.matmul(
                            sc[:, hi, 0:128], tri_sb[:], id_sb[:],
                            start=False, stop=True,
                            skip_group_check=True,
                        )
                ex = ex_pool.tile([128, 2, CH], dt_i, tag="ex", name="ex")
                # timing-only ablations: tinyexp shrinks the ACT op; decouple
                # additionally feeds ctx from a const (no ACT->PE edge)
                nex = 8 if ablate in ("tinyexp", "decouple") else n
                nc.scalar.activation(
                    ex[:, :, 0:nex], sc[:, :, 0:nex],
                    mybir.ActivationFunctionType.Exp, scale=1.0 / np.sqrt(D))
                if diag and mask_dve:
                    nc.vector.tensor_mul(
                        ex[:, :, 0:128], ex[:, :, 0:128],
                        trim_sb.unsqueeze(1).broadcast_to([128, 2, 128]))
                if ablate == "decouple":
                    ex = exc_sb
                # ACT-vs-PE deficit for this tile: exp+overheads minus the
                # scores/ctx matmul time; pop that much filler PE work
                deficit += (2 * n * 0.833 + 356) - (4 * n * 0.4167)
                while fillers and filled < deficit:
                    cost, f = fillers.pop(0)
                    f()
                    filled += cost
                # delay ctx by ctx_delay k-tiles: later scores issue on the
                # PE before ctx(kt), hiding the exp latency
                ctx_q.append((kt, ex, c0, n))
                if len(ctx_q) > ctx_delay:
                    emit_ctx(*ctx_q.pop(0))
            for args in ctx_q:
                emit_ctx(*args)
            for _, f in fillers:  # drain leftovers
                f()
            ctxn = cn_pool.tile([128, CH], dt_i, tag="ctxn", name="ctxn")
            return cps, ctxn

        if ablate == "projonly":
            # timing skeleton: projections + out-proj only, no attention
            for qc in range(ncb):
                xts = alloc_x(qc)
                ctxns = []
                for p in range(NP):
                    fl, _ = proj_fillers(qc, p, xts)
                    for _, f in fl:
                        f()
                    ctxn = cn_pool.tile([128, CH], dt_i, tag="ctxn",
                                        name="ctxn")
                    nc.vector.memset(ctxn[:], 0.5)
                    ctxns.append(ctxn)
                for _, f in outproj_fillers(qc, ctxns):
                    f()
        elif ablate == "attnonly":
            # timing skeleton: attention only; q/k/v from consts, no proj/IO
            kc = consts.tile([128, seq], dt_i, name="kc")
            nc.vector.memset(kc[:], 0.01)
            vc = consts.tile([128, nkt_b, 130], dt_i, name="vc")
            nc.vector.memset(vc[:], 0.01)
            qc_t = consts.tile([128, CH], dt_i, name="qc_t")
            nc.vector.memset(qc_t[:], 0.01)
            kTs[:] = [kc] * NP
            vsbs[:] = [vc] * NP
            for qc in range(ncb):
                for p in range(NP):
                    cps, ctxn = attention(qc, p, qc_t, [])
                    for _, f in norm_fillers(cps, ctxn):
                        f()
        else:
            pairs = [(qc, p) for qc in range(ncb) for p in range(NP)]
            xts_by = {0: alloc_x(0)}
            first, qTt_next = proj_fillers(0, 0, xts_by[0])
            for _, f in first:
                f()
            pending = None       # ctxn tiles of prev chunk awaiting out-proj
            pending_norm = None  # (cps, ctxn) of prev pair, not normalized
            ctxns = []
            for idx, (qc, p) in enumerate(pairs):
                qTt = qTt_next
                fillers = []
                if pending_norm is not None:
                    fillers += norm_fillers(*pending_norm)
                if (qc, p) != pairs[-1]:
                    nqc, npp = pairs[idx + 1]
                    if npp == 0:
                        xts_by[nqc] = alloc_x(nqc)
                        xts_by.pop(nqc - 1, None)
                    nf, qTt_next = proj_fillers(nqc, npp, xts_by[nqc])
                    fillers += nf
                if pending is not None:
                    half = CH // 256  # out-proj t-tiles split across pairs
                    if p == 0:
                        fillers += outproj_fillers(qc - 1, pending, 0, half)
                    else:
                        fillers += outproj_fillers(qc - 1, pending, half,
                                                   CH // 128)
                        pending = None
                cps, ctxn = attention(qc, p, qTt, fillers)
                pending_norm = (cps, ctxn)
                ctxns.append(ctxn)
                if p == NP - 1:
                    pending = ctxns
                    ctxns = []

            for _, f in norm_fillers(*pending_norm):
                f()
            pending_norm = None
            for _, f in outproj_fillers(ncb - 1, pending):
                f()
            pending = None

        if rep_cm is not None:
            rep_cm.__exit__(None, None, None)

    nc.compile()
    return nc


def _prep_inputs(x, Wq, Wk, Wv, Wo, dt_in=np.float32):
    """Host-side sharding: transpose x per batch, slice weights per core."""
    batch, seq, _ = x.shape
    xTs = [np.ascontiguousarray(x[b].T).astype(dt_in) for b in range(batch)]
    # trineg[q, k] = -30000 iff k > q; used as lhsT with rhs=I so the scores
    # PSUM group accumulates -30000 into causally-invalid diagonal entries
    trineg = -30000.0 * np.triu(np.ones((128, 128), np.float32), 1)
    identity = np.eye(128, dtype=np.float32)

    def warr(w):  # [E, 128] col-slice -> SBUF layout [128, 8*128]
        return np.ascontiguousarray(
            w.reshape(NE, 128, 128).transpose(1, 0, 2).reshape(128, E)
        ).astype(dt_in)

    in_maps = []
    for i in range(N_CORES):
        b, g = i // NG, i % NG
        cs = [slice(g * 256 + p * 128, g * 256 + (p + 1) * 128)
              for p in range(NP)]
        in_maps.append({
            "xT": xTs[b],
            "wq": np.concatenate([warr(Wq[:, c]) for c in cs], axis=0),
            "wk": np.concatenate([warr(Wk[:, c]) for c in cs], axis=0),
            "wv": np.concatenate([warr(Wv[:, c]) for c in cs], axis=0),
            "wo": np.concatenate(
                [np.ascontiguousarray(Wo[c, :]) for c in cs], axis=0
            ).astype(dt_in),
            "trineg": trineg.astype(dt_in),
            "iden": identity.astype(dt_in),
            "trimask": np.triu(np.ones((128, 128), np.float32)).astype(dt_in),
            "onesc": np.ones((128, 1), dt_in),
        })
    return in_maps


_CACHE = {}


def _get_nc(batch, seq, dt_in, dt_out):
    key = (batch, seq, dt_in, dt_out)
    if key not in _CACHE:
        _CACHE[key] = build_attention(batch, seq, dt_in, dt_out)
    return _CACHE[key]


DT_IN = F16   # fp16 x/W transfers; projections accumulate fp32 in PSUM
DT_OUT = F16  # fp16 partial-y transfers; host sums in fp32


def kernel(x, Wq, Wk, Wv, Wo, bo, _trace=False):
    x = np.asarray(x, np.float32)
    batch, seq, _ = x.shape
    nc = _get_nc(batch, seq, DT_IN, DT_OUT)
    in_maps = _prep_inputs(x, np.asarray(Wq), np.asarray(Wk), np.asarray(Wv),
                           np.asarray(Wo),
                           dt_in=np.float16 if DT_IN == F16 else np.float32)
    res = run_bass_kernel_spmd(nc, in_maps, core_ids=list(range(N_CORES)),
                               trace=_trace)
    y = np.empty((batch, seq, E), np.float32)
    bo32 = np.asarray(bo, np.float32)
    for b in range(NB):
        parts = [res.results[b * NG + g]["y"].astype(np.float32)
                 for g in range(NG)]
        y[b] = np.sum(parts, axis=0, dtype=np.float32) + bo32
    if _trace:
        kernel.last_results = res
    return y
